# revision 2
# baseline (speedup 1.0000x reference)
"""Trainium2 Bass kernel for nn_Attention_38233798869191 (v2, all-bf16).

Full multi-head attention layer (B=2, S=2048, D=1024, H=16, dh=64) with the
reference's "faithful to original" reshape quirk, sharded over 8 NeuronCores
by splitting heads (tensor parallel): core c owns heads {2c, 2c+1}.

Differences vs v1 (f32r):
  * bf16 datapath end to end (x, weights, q/k, p, v, M, y-partials); all
    PSUM accumulation stays f32. Halves HBM traffic and removes the f32r
    small-free-dim matmul penalty.
  * v is produced directly in NATURAL layout [seq, feat] by swapping matmul
    operand roles (lhsT = x tile, rhs = Wv tile) -- no TensorE transposes,
    no per-chunk DVE copies.
  * p@v runs in natural-o form: out[sq 128, 65] accumulated over key chunks
    with lhsT = p-tile chunk, rhs = v-natural slice with a ones column, so
    column 64 accumulates the softmax denominator. Costs 65 PE rows per
    chunk instead of the 512-row transposed form and kills the
    transpose-back + PSUM->SBUF copy of o.
  * biases are applied with K=1 matmuls (bias row x ones row) accumulated
    into the projection PSUM group, so PSUM evictions are plain DVE copies
    and ACT does nothing but the 128 exp instructions per rep.
  * software-pipelined across reps: rep r's A(1,1) hides rep r+1's batch-0
    projections; rep r's output projection for batch 1 hides inside rep
    r+1's A(0,0).

Per-core dataflow:
  xT [1024, 4096] bf16 (host-pretransposed x, shared by all cores)
  qT = WqT_c.T @ xT (SCALE folded into Wq host-side)  [128, 4096]; kT same.
  v natural per batch: v2[b] [sk 128, kc 16, head 2, 65], column 64 == 1.0
  (persistent memset; evictions only write cols 0:64).
  Per (b, head, 1024-query half): for each of 16 key chunks,
  scoresT[sk 128, sq 1024] = kT.T @ qT (PSUM), p = exp(scoresT) -> bf16
  SBUF (no max-subtraction: scores are O(1) by construction); o-chunks
  [sq 128, 65] += p_chunk.T @ v2-slice accumulated in PSUM over key chunks;
  normalize by the denominator column (DVE reciprocal + tensor-scalar mul)
  -> o natural bf16 -> DRAM scratch; M rows DMA'd back as [64, 2048] per
  head (the reference's o.reshape(B, D, S) quirk makes the output
  projection y[b].T = Wo[:, c*128:(c+1)*128] @ M_c[b] where M_c[b] is o
  natural reinterpreted as [64, 2048] per head). Partial y[b].T written
  bf16; summed over cores on the host (the tensor-parallel all-reduce) and
  transposed back.
"""

import os
import sys

import numpy as np

for _p in ("/opt/trn_rl_repo", "/root/.axon_site/_ro/trn_rl_repo"):
    if os.path.isdir(_p) and _p not in sys.path:
        sys.path.insert(0, _p)

B, S, D, H, DH = 2, 2048, 1024, 16, 64
NSEQ = B * S  # 4096
SCALE = 1.0 / float(np.sqrt(DH))
N_CORES = 8
P = 128

DTYPE_MODE = os.environ.get("KERNEL_DTYPE_MODE", "bf16")


def _build_nc(mode="bf16", reps=1):
    import concourse.bass as bass  # noqa: F401
    import concourse.mybir as mybir
    import concourse.tile as tile
    from concourse import bacc

    f32 = mybir.dt.float32
    elt = mybir.dt.bfloat16
    AF = mybir.ActivationFunctionType

    nc = bacc.Bacc(
        "TRN2",
        target_bir_lowering=False,
        debug=False,
        num_devices=N_CORES,
    )

    xT = nc.dram_tensor("xT", [D, NSEQ], elt, kind="ExternalInput")
    wqT = nc.dram_tensor("wqT", [D, P], elt, kind="ExternalInput")
    wkT = nc.dram_tensor("wkT", [D, P], elt, kind="ExternalInput")
    wvT = nc.dram_tensor("wvT", [D, P], elt, kind="ExternalInput")
    woT = nc.dram_tensor("woT", [P, D], elt, kind="ExternalInput")
    bqc = nc.dram_tensor("bqc", [P, 1], f32, kind="ExternalInput")  # bq*SCALE
    bkc = nc.dram_tensor("bkc", [P, 1], f32, kind="ExternalInput")
    bvr = nc.dram_tensor("bvr", [1, P], elt, kind="ExternalInput")
    ypT = nc.dram_tensor("ypT", [B, D, S], elt, kind="ExternalOutput")
    osc = nc.dram_tensor("osc", [2 * 2, S, DH], elt)  # o natural per (b, hl)

    # DRAM views
    # d global = ko*512 + ks*128 + p
    xTv = xT.ap().rearrange("(ko ks p) s -> ko p ks s", ks=4, p=P)
    oscM = osc.ap().rearrange("h (r k) d -> h r (k d)", r=64)  # [4, 64, 2048]

    def wview(w):
        return w.ap().rearrange("(kc p) m -> p kc m", p=P)  # [128, 8, 128]

    with tile.TileContext(nc) as tc:
        with tc.tile_pool(name="persist", bufs=1) as pp:
            w_sb = {}
            for name, w in (("q", wqT), ("k", wkT), ("v", wvT)):
                w_sb[name] = pp.tile([P, 8, P], elt, tag=f"w{name}", name=f"w{name}")
                nc.sync.dma_start(w_sb[name][:], wview(w))
            woT_sb = pp.tile([P, D], elt, tag="wo", name="wo")
            nc.sync.dma_start(woT_sb[:], woT.ap())
            # q/k biases as per-partition columns (added during DVE eviction);
            # v bias as a row (K=1 matmul into the natural-layout group)
            bias_sb = {}
            for name, bt, shape, dt_ in (
                ("q", bqc, [P, 1], f32),
                ("k", bkc, [P, 1], f32),
                ("v", bvr, [1, P], elt),
            ):
                bias_sb[name] = pp.tile(shape, dt_, tag=f"b{name}", name=f"b{name}")
                nc.sync.dma_start(bias_sb[name][:], bt.ap())
            ones_sb = pp.tile([1, 512], elt, tag="ones", name="ones")
            nc.vector.memset(ones_sb[:], 1.0)

            qT_sb = pp.tile([P, NSEQ], elt, tag="qT", name="qT")
            kT_sb = pp.tile([P, NSEQ], elt, tag="kT", name="kT")
            # v natural per batch: [sk 128, kc 16, head 2, 65]; col 64 == 1.0
            # forever (evictions only write cols 0:64) -> the p@v matmul's
            # 65th output column accumulates the softmax denominator.
            v2 = [
                pp.tile([P, 16, 2, 65], elt, tag=f"v{b}", name=f"v{b}")
                for b in range(B)
            ]
            for b in range(B):
                nc.vector.memset(v2[b][:], 1.0)
            M_sb = [pp.tile([P, S], elt, tag=f"M{b}", name=f"M{b}") for b in range(B)]

            with (
                tc.tile_pool(name="xin", bufs=6) as xpool,
                # PSUM budget (8 banks), one accumulation group per bank:
                #   ps:  2 x [128,1024] f32 = 4 banks (scores)
                #   acc: 2 x [128,512] f32  = 2 banks (p@v accumulators)
                #   aux: 2 x [128,512] f32  = 2 banks (q/k/v-nat/outproj)
                tc.tile_pool(name="psum", bufs=1, space="PSUM") as psp,
                tc.tile_pool(name="ptp", bufs=24) as ptp,
                tc.tile_pool(name="obp", bufs=2) as obp,
                tc.tile_pool(name="rcp", bufs=4) as rcp,
                tc.tile_pool(name="ysb", bufs=4) as ysbp,
            ):
                _live = {}

                def load_x(sq):
                    tiles = []
                    for ko in range(2):
                        x_sb = xpool.tile([P, 4, 512], elt, tag="x", name="x")
                        for xh in range(2):
                            nc.sync.dma_start(
                                x_sb[:, xh * 2 : (xh + 1) * 2, :],
                                xTv[
                                    ko,
                                    :,
                                    xh * 2 : (xh + 1) * 2,
                                    sq * 512 : (sq + 1) * 512,
                                ],
                            )
                        tiles.append(x_sb)
                    _live[("x", sq)] = tiles

                def qk_mm(sq, n):
                    """q or k projection for one 512-column slab: 8-step
                    accumulation into a 1-bank PSUM tile; bias applied by the
                    DVE eviction (tensor_scalar_add with per-partition col)."""
                    pacc = psp.tile([P, 512], f32, tag="aux", bufs=2, name="aux")
                    x_tiles = _live[("x", sq)]
                    for ko in range(2):
                        for ks in range(4):
                            nc.tensor.matmul(
                                pacc[:],
                                w_sb[n][:, ko * 4 + ks, :],
                                x_tiles[ko][:, ks, :],
                                start=(ko == 0 and ks == 0),
                                stop=(ko == 1 and ks == 3),
                            )
                    dst = qT_sb if n == "q" else kT_sb
                    nc.vector.tensor_scalar_add(
                        dst[:, sq * 512 : (sq + 1) * 512], pacc[:], bias_sb[n][:]
                    )

                def v_mm(sq, bi, mh):
                    """v natural for sub-chunks {2*mh, 2*mh+1} of this slab."""
                    if ("sv", sq) not in _live:
                        _live[("sv", sq)] = psp.tile(
                            [P, 512], f32, tag="aux", bufs=2, name="aux"
                        )
                    sv = _live[("sv", sq)]
                    x_tiles = _live[("x", sq)]
                    for m in (2 * mh, 2 * mh + 1):
                        svm = sv[:, m * P : (m + 1) * P]
                        for ko in range(2):
                            for ks in range(4):
                                nc.tensor.matmul(
                                    svm,
                                    x_tiles[ko][:, ks, m * P : (m + 1) * P],
                                    w_sb["v"][:, ko * 4 + ks, :],
                                    start=(ko == 0 and ks == 0),
                                    stop=False,
                                )
                        nc.tensor.matmul(
                            svm,
                            ones_sb[:, 0:P],
                            bias_sb["v"][:],
                            start=False,
                            stop=True,
                        )
                    if mh == 1:
                        sv = _live.pop(("sv", sq))
                        _live.pop(("x", sq))
                        kc0 = (sq - bi * 4) * 4
                        nc.vector.tensor_copy(
                            v2[bi][:, kc0 : kc0 + 4, :, 0:64],
                            sv[:].rearrange("p (m h x) -> p m h x", m=4, h=2),
                        )

                def proj_units(bi):
                    """Batch bi's projections as (weight, fn) units; x loads
                    prefetched two slabs ahead of their consumers."""
                    slabs = list(range(bi * 4, bi * 4 + 4))
                    units = [(0.1, lambda sq=sq: load_x(sq)) for sq in slabs[:2]]
                    for i, sq in enumerate(slabs):
                        units.append((1.7, lambda sq=sq: qk_mm(sq, "q")))
                        units.append((1.7, lambda sq=sq: qk_mm(sq, "k")))
                        units.append((1.2, lambda sq=sq: v_mm(sq, bi, 0)))
                        units.append((1.2, lambda sq=sq: v_mm(sq, bi, 1)))
                        if i + 2 < 4:
                            units.append(
                                (0.1, lambda sq=slabs[i + 2]: load_x(sq))
                            )
                    return units

                def outproj_unit(b, mo, j, evict_engine):
                    if j == 0:
                        _live[("y", b, mo)] = ysbp.tile(
                            [P, 2048], elt, tag="y", name="y"
                        )
                    ysb = _live[("y", b, mo)]
                    py = psp.tile([P, 512], f32, tag="aux", bufs=2, name="aux")
                    nc.tensor.matmul(
                        py[:],
                        woT_sb[:, mo * P : (mo + 1) * P],
                        M_sb[b][:, j * 512 : (j + 1) * 512],
                        start=True,
                        stop=True,
                    )
                    dst = ysb[:, j * 512 : (j + 1) * 512]
                    if evict_engine == "act":
                        nc.scalar.copy(dst, py[:])
                    else:
                        nc.vector.tensor_copy(dst, py[:])
                    if j == 3:
                        _live.pop(("y", b, mo))
                        nc.sync.dma_start(
                            ypT.ap()[b, mo * P : (mo + 1) * P, :], ysb[:]
                        )

                def outproj_units(b, evict_engine="dve"):
                    return [
                        (0.45, lambda mo=mo, j=j: outproj_unit(b, mo, j, evict_engine))
                        for mo in range(8)
                        for j in range(4)
                    ]

                pace_s1 = float(os.environ.get("PACE_S1", "0.35"))
                pace_s2 = float(os.environ.get("PACE_S2", "0.85"))
                pace_cap = float(os.environ.get("PACE_CAP", "1.8"))
                w_scale = float(os.environ.get("PACE_W", "1.0"))
                pace_deep = float(os.environ.get("PACE_DEEP", "1e9"))
                pace_boost = float(os.environ.get("PACE_BOOST", "1.5"))

                class Pacer:
                    """Credit-based spreading of filler PE work into the
                    ACT-bound attention stream: each slot adds `budget` us of
                    PE headroom; units run when their estimated PE time fits
                    the accumulated credit."""

                    def __init__(self):
                        self.q = []
                        self.credit = 0.0

                    def add(self, units):
                        self.q.extend(units)

                    def slot(self, budget):
                        backlog = sum(w for w, _ in self.q)
                        if backlog > pace_deep:
                            budget *= pace_boost
                        self.credit = min(self.credit + budget, pace_cap)
                        while self.q and self.q[0][0] * w_scale <= self.credit:
                            w, fn = self.q.pop(0)
                            self.credit -= w * w_scale
                            fn()

                    def flush(self):
                        for _, fn in self.q:
                            fn()
                        self.q.clear()
                        self.credit = 0.0

                def attention_head(b, hl, pacer):
                    bh = b * 2 + hl
                    hsl = slice(hl * 64, (hl + 1) * 64)
                    for sqh in range(2):  # halves of 1024 queries
                        sq0 = b * S + sqh * 1024
                        # stage 1: scores + exp for all 16 key chunks (the
                        # p tiles all stay live through stage 2)
                        ptiles = []
                        for kc in range(16):
                            k0 = b * S + kc * P
                            ps = psp.tile([P, 1024], f32, tag="ps", bufs=2, name="ps")
                            for half in range(2):
                                nc.tensor.matmul(
                                    ps[:, half * 512 : (half + 1) * 512],
                                    kT_sb[hsl, k0 : k0 + P],
                                    qT_sb[
                                        hsl,
                                        sq0 + half * 512 : sq0 + (half + 1) * 512,
                                    ],
                                    start=True,
                                    stop=True,
                                )
                            ptile = ptp.tile([P, 1024], elt, tag="pt", name="pt")
                            nc.scalar.activation(ptile[:], ps[:], AF.Exp)
                            ptiles.append(ptile)
                            pacer.slot(pace_s1)
                        # stage 2: p@v chunk-major -- one accumulation group
                        # at a time per PSUM bank (zero-region rule)
                        ob = obp.tile([P, 8, DH], elt, tag="ob", name="ob")
                        for ch in range(8):
                            acc = psp.tile([P, 512], f32, tag="acc", bufs=2, name="acc")
                            for kc in range(16):
                                nc.tensor.matmul(
                                    acc[:, 0:65],
                                    ptiles[kc][:, ch * P : (ch + 1) * P],
                                    v2[b][:, kc, hl, :],
                                    start=(kc == 0),
                                    stop=(kc == 15),
                                )
                            rc = rcp.tile([P, 1], f32, tag="rc", name="rc")
                            nc.vector.reciprocal(rc[:], acc[:, 64:65])
                            nc.vector.tensor_scalar_mul(
                                ob[:, ch, :], acc[:, 0:64], rc[:]
                            )
                            pacer.slot(pace_s2)
                        s0 = sqh * 1024
                        nc.sync.dma_start(
                            osc.ap()[bh, s0 : s0 + 1024, :].rearrange(
                                "(t p) d -> p t d", p=P
                            ),
                            ob[:],
                        )
                        # M rows covered by this half (32 of the head's 64)
                        r0 = hl * 64 + sqh * 32
                        nc.sync.dma_start(
                            M_sb[b][r0 : r0 + 32, :],
                            oscM[bh][sqh * 32 : (sqh + 1) * 32],
                        )

                # software-pipelined rep schedule:
                #   prologue: P(0) of rep 0
                #   rep r: A(0,0)+[O(1)_{r-1}, P(1)_r] ; A(0,1)+leftover ;
                #          A(1,0)+[O(0)_r] ; A(1,1)+[P(0)_{r+1}]
                #   epilogue: O(1) of last rep
                pacer = Pacer()
                for _, u in proj_units(0):
                    u()
                for rep in range(reps):
                    if rep > 0:
                        pacer.add(outproj_units(1, "dve"))
                    pacer.add(proj_units(1))
                    attention_head(0, 0, pacer)
                    attention_head(0, 1, pacer)
                    pacer.add(outproj_units(0, "dve"))
                    attention_head(1, 0, pacer)
                    if rep < reps - 1:
                        pacer.add(proj_units(0))
                    attention_head(1, 1, pacer)
                    pacer.flush()
                for _, u in outproj_units(1, "act"):
                    u()

    nc.compile()
    return nc


_CACHE = {}


def _np_elt(mode=None):
    import ml_dtypes

    return ml_dtypes.bfloat16


def _get_runner(mode, reps=1):
    """Build (once) the compiled kernel + a persistent jitted executor."""
    key = (mode, reps)
    if key in _CACHE:
        return _CACHE[key]

    import jax
    import jax.numpy as jnp  # noqa: F401
    from jax.sharding import Mesh, PartitionSpec
    from jax.experimental.shard_map import shard_map
    import concourse.mybir as mybir
    from concourse import bass2jax

    nc = _build_nc(mode, reps)
    bass2jax.install_neuronx_cc_hook()

    partition_name = (
        nc.partition_id_tensor.name if nc.partition_id_tensor else None
    )
    in_names = []
    out_names = []
    out_avals = []
    for alloc in nc.m.functions[0].allocations:
        if not isinstance(alloc, mybir.MemoryLocationSet):
            continue
        name = alloc.memorylocations[0].name
        if alloc.kind == "ExternalInput":
            if name != partition_name:
                in_names.append(name)
        elif alloc.kind == "ExternalOutput":
            out_names.append(name)
            shape = tuple(alloc.tensor_shape)
            dtype = mybir.dt.np(alloc.dtype)
            out_avals.append(jax.core.ShapedArray(shape, dtype))
    n_params = len(in_names)
    n_outs = len(out_avals)
    all_in_names = list(in_names) + list(out_names)
    if partition_name is not None:
        all_in_names.append(partition_name)
    all_in_names = tuple(all_in_names)

    def _body(*args):
        operands = list(args)
        if partition_name is not None:
            operands.append(bass2jax.partition_id_tensor())
        outs = bass2jax._bass_exec_p.bind(
            *operands,
            out_avals=tuple(out_avals),
            in_names=all_in_names,
            out_names=tuple(out_names),
            lowering_input_output_aliases=(),
            sim_require_finite=True,
            sim_require_nnan=True,
            nc=nc,
        )
        return tuple(outs)

    devices = jax.devices()[:N_CORES]
    mesh = Mesh(np.asarray(devices), ("core",))
    in_specs = (PartitionSpec("core"),) * (n_params + n_outs)
    out_specs = (PartitionSpec("core"),) * n_outs
    donate = tuple(range(n_params, n_params + n_outs))
    sharded = jax.jit(
        shard_map(
            _body, mesh=mesh, in_specs=in_specs, out_specs=out_specs,
            check_rep=False,
        ),
        donate_argnums=donate,
        keep_unused=True,
    )

    zero_out_shapes = [
        ((N_CORES * a.shape[0],) + tuple(a.shape[1:]), a.dtype)
        for a in out_avals
    ]

    def execute(in_maps):
        concat_in = [
            np.concatenate([np.asarray(m[name]) for m in in_maps], axis=0)
            for name in in_names
        ]
        concat_zeros = [np.zeros(s, d) for s, d in zero_out_shapes]
        out_arrs = sharded(*concat_in, *concat_zeros)
        out_arrs = [np.asarray(o) for o in out_arrs]
        return [
            {
                name: out_arrs[i].reshape(
                    N_CORES, *out_avals[i].shape
                )[c]
                for i, name in enumerate(out_names)
            }
            for c in range(N_CORES)
        ]

    execute.in_names = in_names
    execute.out_names = out_names
    execute.out_avals = out_avals
    execute.n_params = n_params
    execute.body = _body
    execute.mesh = mesh
    execute.zero_out_shapes = zero_out_shapes
    _CACHE[key] = execute
    return execute


def make_in_maps(x, Wq, bq, Wk, bk, Wv, bv, Wo, bo, mode=None):
    ne = _np_elt()
    x = np.asarray(x, np.float32)
    xT = np.ascontiguousarray(x.reshape(NSEQ, D).T).astype(ne)
    in_maps = []
    for c in range(N_CORES):
        sl = slice(c * P, (c + 1) * P)
        in_maps.append(
            {
                "xT": xT,
                "wqT": np.ascontiguousarray(
                    np.asarray(Wq)[sl, :].T * SCALE
                ).astype(ne),
                "wkT": np.ascontiguousarray(np.asarray(Wk)[sl, :].T).astype(ne),
                "wvT": np.ascontiguousarray(np.asarray(Wv)[sl, :].T).astype(ne),
                "woT": np.ascontiguousarray(np.asarray(Wo)[:, sl].T).astype(ne),
                "bqc": np.ascontiguousarray(
                    (np.asarray(bq, np.float32)[sl] * SCALE).reshape(P, 1)
                ),
                "bkc": np.ascontiguousarray(
                    np.asarray(bk, np.float32)[sl].reshape(P, 1)
                ),
                "bvr": np.asarray(bv, np.float32)[sl].reshape(1, P).astype(ne),
            }
        )
    return in_maps


def kernel(x, Wq, bq, Wk, bk, Wv, bv, Wo, bo):
    mode = DTYPE_MODE
    execute = _get_runner(mode)
    in_maps = make_in_maps(x, Wq, bq, Wk, bk, Wv, bv, Wo, bo, mode)
    results = execute(in_maps)
    ysum = np.zeros((B, D, S), np.float64)
    for c in range(N_CORES):
        ysum += np.asarray(results[c]["ypT"], np.float32)
    y = ysum.transpose(0, 2, 1) + np.asarray(bo, np.float32)[None, None, :]
    return np.ascontiguousarray(y.astype(np.float32))
